# revision 3
# baseline (speedup 1.0000x reference)
"""Trainium2 Bass kernel for nn_Conv2d_mvm (PUMA bit-sliced crossbar conv emulation).

Math identity used
------------------
The reference emulates an analog crossbar MVM: inputs are 16-bit two's-complement
bit-streams, weights are 2-bit slices of 16-bit magnitudes of the pos/neg parts,
ADC = clip(round(analog), 0, 511). Per crossbar (128 rows), each analog column sum
is at most 128*3 = 384 < 511, and all quantities are small exact integers in f32,
so ADC is the identity and the whole pipeline is linear in the bits/slices.
Shift-add therefore reconstructs exactly:

    out[p, c] = quant( (x_int[p, :] . w_int[c, :]) / 2^24 )

with x_int = round(patch * 2^12) (int16 wrap), w_int = round(relu(w)*2^12) -
round(relu(-w)*2^12) (each clipped to [0, 65535]), and
quant(v) = clip(round(v * 2^12), -32768, 32767) / 2^12.

Sharding: data-parallel over the P = B*OH*OW = 1024 output pixels; each of the
8 cores computes 128 pixels (half of one batch image) against the full
[L=576, Cout=128] integer weight matrix (replicated).

Device kernel per core: 9 windowed DMAs perform the im2col patch extraction into
5 l-tiles of [128, 128], 5 accumulating f32 matmuls on the PE array produce
acc[pixel, cout] in PSUM, and a 3-instruction DVE epilogue applies the fixed
point quantizer (round-to-nearest-even via the +1.5*2^23 magic constant).
"""

import numpy as np

# Problem constants (hardcoded per contract: kernel.py must be self-contained).
B, CIN, H, W = 4, 64, 16, 16
COUT = 128
K, PAD = 3, 1
IF = 12           # input frac bits
WF = 12           # weight frac bits
ACM_FRAC = 12
L = CIN * K * K   # 576
LP = 640          # padded to 5 * 128
N_CORES = 8
ROWS_PER_CORE = H // 2          # 8 pixel rows per core
PIX_PER_CORE = ROWS_PER_CORE * W  # 128

_CACHE = {}


def _build_program():
    import concourse.bacc as bacc
    import concourse.mybir as mybir
    import concourse.tile as tile

    nc = bacc.Bacc("TRN2", target_bir_lowering=False, debug=False,
                   num_devices=N_CORES)

    xp = nc.dram_tensor("xp", [CIN, ROWS_PER_CORE + 2, W + 2], mybir.dt.float32,
                        kind="ExternalInput").ap()
    wm = nc.dram_tensor("wm", [LP, COUT], mybir.dt.float32,
                        kind="ExternalInput").ap()
    out = nc.dram_tensor("out", [PIX_PER_CORE, COUT], mybir.dt.float32,
                         kind="ExternalOutput").ap()

    MAGIC = float(np.float32(1.5 * 2 ** 23))  # RNE rounding constant
    INV_Q = 1.0 / (1 << ACM_FRAC)
    LO = float(-(1 << 15))
    HI = float((1 << 15) - 1)

    with tile.TileContext(nc) as tc:
        with (
            tc.tile_pool(name="sbuf", bufs=1) as pool,
            tc.tile_pool(name="psum", bufs=1, space="PSUM") as psum_pool,
        ):
            # Full integer weight matrix, 5 l-tiles of [128, 128].
            wt = pool.tile([128, LP // 128, COUT], mybir.dt.float32)
            nc.sync.dma_start(wt[:, :, :],
                              wm.rearrange("(r k) n -> k r n", k=128))

            # Patch tiles: lhsT[r] holds l = 128r..128r+127 on partitions,
            # the core's 128 pixels on the free dim. l ordering is
            # (ki, kj, cin): l = (ki*3 + kj)*64 + cin  (wm built to match).
            xt = [pool.tile([128, ROWS_PER_CORE, W], mybir.dt.float32,
                            name=f"xt{r}", tag=f"x{r}")
                  for r in range(LP // 128)]
            # Tail partitions of the last tile multiply only zero weight rows,
            # but zero them so nothing uninitialized is ever read.
            nc.vector.memset(xt[4][64:128, :, :], 0.0)
            for g in range(K * K):
                ki, kj = divmod(g, K)
                r, half = divmod(g, 2)
                nc.sync.dma_start(
                    xt[r][64 * half:64 * (half + 1), :, :],
                    xp[:, ki:ki + ROWS_PER_CORE, kj:kj + W],
                )

            acc = psum_pool.tile([PIX_PER_CORE, COUT], mybir.dt.float32)
            for r in range(LP // 128):
                nc.tensor.matmul(acc[:, :], xt[r][:, :, :], wt[:, r, :],
                                 start=(r == 0), stop=(r == LP // 128 - 1))

            res = pool.tile([PIX_PER_CORE, COUT], mybir.dt.float32)
            # q = clip(round(acc / 2^12), -2^15, 2^15-1) / 2^12, RNE rounding.
            nc.vector.tensor_scalar(res[:, :], acc[:, :], INV_Q, MAGIC,
                                    op0=mybir.AluOpType.mult,
                                    op1=mybir.AluOpType.add)
            nc.vector.tensor_scalar(res[:, :], res[:, :], MAGIC, LO,
                                    op0=mybir.AluOpType.subtract,
                                    op1=mybir.AluOpType.max)
            nc.vector.tensor_scalar(res[:, :], res[:, :], HI, INV_Q,
                                    op0=mybir.AluOpType.min,
                                    op1=mybir.AluOpType.mult)
            nc.sync.dma_start(out, res[:, :])

    nc.compile()
    return nc


def _quantize_inputs(x, w):
    """Reproduce the reference's fixed-point quantization bit-exactly."""
    # x_int = round(x * 2^IF) as int32, wrapped to int16 two's complement
    # (the reference masks to 16 bits and uses bit 15 as the sign).
    xi = np.round(x.astype(np.float32) * (1 << IF)).astype(np.int64)
    xi = ((xi + (1 << 15)) & 0xFFFF) - (1 << 15)

    wf = w.reshape(COUT, L).astype(np.float32)
    w_pos = np.clip(np.round(np.clip(wf, 0, None) * (1 << WF)), 0, 65535)
    w_neg = np.clip(np.round(np.abs(np.clip(wf, None, 0)) * (1 << WF)), 0, 65535)
    wi = (w_pos - w_neg).astype(np.int64)  # [COUT, L], l = (cin, ki, kj)
    return xi, wi


def kernel(x: np.ndarray, w: np.ndarray) -> np.ndarray:
    from concourse.bass_utils import run_bass_kernel_spmd

    x = np.asarray(x, dtype=np.float32)
    w = np.asarray(w, dtype=np.float32)

    xi, wi = _quantize_inputs(x, w)

    # Padded quantized input images, stored as f32 integers: [B, CIN, H+2, W+2]
    xpad = np.zeros((B, CIN, H + 2, W + 2), dtype=np.float32)
    xpad[:, :, PAD:PAD + H, PAD:PAD + W] = xi.astype(np.float32)

    # Weight matrix in (ki, kj, cin) l-order: wm[(ki*3+kj)*64 + cin, cout]
    wm = np.zeros((LP, COUT), dtype=np.float32)
    wmat = wi.reshape(COUT, CIN, K, K).transpose(2, 3, 1, 0).reshape(L, COUT)
    wm[:L, :] = wmat.astype(np.float32)

    if "nc" not in _CACHE:
        _CACHE["nc"] = _build_program()
    nc = _CACHE["nc"]

    in_maps = []
    for core in range(N_CORES):
        b, half = divmod(core, 2)
        r0 = half * ROWS_PER_CORE
        in_maps.append({
            "xp": np.ascontiguousarray(xpad[b, :, r0:r0 + ROWS_PER_CORE + 2, :]),
            "wm": wm,
        })

    results = run_bass_kernel_spmd(nc, in_maps, list(range(N_CORES))).results

    out = np.empty((B, COUT, H, W), dtype=np.float32)
    for core in range(N_CORES):
        b, half = divmod(core, 2)
        r0 = half * ROWS_PER_CORE
        shard = results[core]["out"].reshape(ROWS_PER_CORE, W, COUT)
        out[b, :, r0:r0 + ROWS_PER_CORE, :] = shard.transpose(2, 0, 1)
    return out


# revision 6
# speedup vs baseline: 1.2012x; 1.2012x over previous
"""Trainium2 Bass kernel for nn_Conv2d_mvm (PUMA bit-sliced crossbar conv emulation).

Math identity
-------------
The reference emulates an analog crossbar MVM: inputs become 16-bit
two's-complement bit-streams, weights become 2-bit slices of the 16-bit
magnitudes of their pos/neg parts, and ADC = clip(round(analog), 0, 511).
Each analog column sum is at most 128*3 = 384 < 511 and every quantity is a
small exact integer held in f32, so the ADC is the identity and the whole
pipeline is linear in the bits/slices. Shift-add therefore reconstructs

    out[p, c] = quant( (x_int[p, :] . w_int[c, :]) / 2^24 )

with x_int = round(patch * 2^12) (int16 wrap),
w_int = clip(round(relu(w)*2^12), 0, 65535) - clip(round(relu(-w)*2^12), 0, 65535),
quant(v) = clip(round(v * 2^12), -2^15, 2^15-1) / 2^12  (round-half-even).

Device kernel
-------------
Data-parallel over the P = 1024 output pixels: each of 8 cores computes 128
pixels (half of one batch image) against the replicated [L=576, Cout=128]
integer weight matrix.

The PE's fp32 matmul is double-pumped (LOW/HIGH passes), so the integer
matmul is run in fp16 instead, which is exact here: |w_int| < 2048 fits
fp16's 11-bit mantissa, and x_int = 256*xh + xl splits into two fp16-exact
factors. The split folds into the contraction dimension,

    acc[p,c] = sum_l (256*xh[l,p]) * w[l,c] + xl[l,p] * w[l,c],

giving k = 2L = 1152 = 9 exact tiles of 128: a single accumulation group of
9 single-pass fp16 matmuls into one PSUM bank. A 3-instruction DVE epilogue
applies the fixed-point quantizer (RNE via the +1.5*2^23 magic constant,
clip, rescale). Should some input exceed the fp16-exact ranges, kernel()
falls back to an fp32 program (5 double-pumped matmuls over k = 640).
"""

import numpy as np

# Problem constants (hardcoded: kernel.py must be self-contained).
B, CIN, H, W = 4, 64, 16, 16
COUT = 128
K, PAD = 3, 1
IF = 12           # input frac bits
WF = 12           # weight frac bits
ACM_FRAC = 12
L = CIN * K * K   # 576
N_CORES = 8
ROWS_PER_CORE = H // 2            # 8 pixel rows per core
PIX_PER_CORE = ROWS_PER_CORE * W  # 128
KT16 = 2 * L // 128               # 9 fp16 k-tiles
KT32 = 5                          # fp32 k-tiles (640 = 5*128, zero-padded)

_CACHE = {}

_MAGIC = float(np.float32(1.5 * 2 ** 23))  # f32 RNE rounding constant
_INV_Q = 1.0 / (1 << ACM_FRAC)
_LO = float(-(1 << 15))
_HI = float((1 << 15) - 1)


def _epilogue_and_out(nc, mybir, pool, acc, out):
    """q = clip(round(acc / 2^12), -2^15, 2^15-1) / 2^12, then store."""
    res = pool.tile([PIX_PER_CORE, COUT], mybir.dt.float32, name="res")
    nc.vector.tensor_scalar(res[:, :], acc[:, :], _INV_Q, _MAGIC,
                            op0=mybir.AluOpType.mult,
                            op1=mybir.AluOpType.add)
    nc.vector.tensor_scalar(res[:, :], res[:, :], _MAGIC, _LO,
                            op0=mybir.AluOpType.subtract,
                            op1=mybir.AluOpType.max)
    nc.vector.tensor_scalar(res[:, :], res[:, :], _HI, _INV_Q,
                            op0=mybir.AluOpType.min,
                            op1=mybir.AluOpType.mult)
    nc.sync.dma_start(out, res[:, :])


def _build_fp16_program():
    """9 single-pass fp16 matmuls; inputs are pre-tiled [9, 128, 128] arrays."""
    import concourse.bacc as bacc
    import concourse.mybir as mybir
    import concourse.tile as tile

    nc = bacc.Bacc("TRN2", target_bir_lowering=False, debug=False,
                   num_devices=N_CORES)
    xk = nc.dram_tensor("xk", [KT16, 128, PIX_PER_CORE], mybir.dt.float16,
                        kind="ExternalInput").ap()
    wk = nc.dram_tensor("wk", [KT16, 128, COUT], mybir.dt.float16,
                        kind="ExternalInput").ap()
    out = nc.dram_tensor("out", [PIX_PER_CORE, COUT], mybir.dt.float32,
                         kind="ExternalOutput").ap()

    with tile.TileContext(nc) as tc:
        with (
            tc.tile_pool(name="sbuf", bufs=1) as pool,
            tc.tile_pool(name="psum", bufs=1, space="PSUM") as psum_pool,
        ):
            xt = pool.tile([128, KT16, PIX_PER_CORE], mybir.dt.float16, name="xt")
            wt = pool.tile([128, KT16, COUT], mybir.dt.float16, name="wt")
            # Two large DMAs on different queues so their issue overlaps.
            nc.sync.dma_start(xt[:, :, :], xk.rearrange("r k p -> k r p"))
            nc.gpsimd.dma_start(wt[:, :, :], wk.rearrange("r k n -> k r n"))

            acc = psum_pool.tile([PIX_PER_CORE, COUT], mybir.dt.float32,
                                 name="acc")
            for r in range(KT16):
                nc.tensor.matmul(acc[:, :], xt[:, r, :], wt[:, r, :],
                                 start=(r == 0), stop=(r == KT16 - 1))
            _epilogue_and_out(nc, mybir, pool, acc, out)

    nc.compile()
    return nc


def _build_fp32_program():
    """Fallback: 5 double-pumped fp32 matmuls over zero-padded k = 640."""
    import concourse.bacc as bacc
    import concourse.mybir as mybir
    import concourse.tile as tile

    nc = bacc.Bacc("TRN2", target_bir_lowering=False, debug=False,
                   num_devices=N_CORES)
    xk = nc.dram_tensor("xk", [KT32, 128, PIX_PER_CORE], mybir.dt.float32,
                        kind="ExternalInput").ap()
    wk = nc.dram_tensor("wk", [KT32, 128, COUT], mybir.dt.float32,
                        kind="ExternalInput").ap()
    out = nc.dram_tensor("out", [PIX_PER_CORE, COUT], mybir.dt.float32,
                         kind="ExternalOutput").ap()

    with tile.TileContext(nc) as tc:
        with (
            tc.tile_pool(name="sbuf", bufs=1) as pool,
            tc.tile_pool(name="psum", bufs=1, space="PSUM") as psum_pool,
        ):
            xt = pool.tile([128, KT32, PIX_PER_CORE], mybir.dt.float32, name="xt")
            wt = pool.tile([128, KT32, COUT], mybir.dt.float32, name="wt")
            nc.sync.dma_start(xt[:, :, :], xk.rearrange("r k p -> k r p"))
            nc.gpsimd.dma_start(wt[:, :, :], wk.rearrange("r k n -> k r n"))

            acc = psum_pool.tile([PIX_PER_CORE, COUT], mybir.dt.float32,
                                 name="acc")
            for r in range(KT32):
                nc.tensor.matmul(acc[:, :], xt[:, r, :], wt[:, r, :],
                                 start=(r == 0), stop=(r == KT32 - 1))
            _epilogue_and_out(nc, mybir, pool, acc, out)

    nc.compile()
    return nc


def _quantize_inputs(x, w):
    """Reproduce the reference's fixed-point quantization bit-exactly."""
    xi = np.round(x.astype(np.float32) * (1 << IF)).astype(np.int64)
    xi = ((xi + (1 << 15)) & 0xFFFF) - (1 << 15)  # int16 two's-complement wrap

    wf = w.reshape(COUT, L).astype(np.float32)
    w_pos = np.clip(np.round(np.clip(wf, 0, None) * (1 << WF)), 0, 65535)
    w_neg = np.clip(np.round(np.abs(np.clip(wf, None, 0)) * (1 << WF)), 0, 65535)
    wi = (w_pos - w_neg).astype(np.int64)  # [COUT, L], l = (cin, ki, kj)
    return xi, wi


def _im2col(xi):
    """[B, CIN, H, W] int -> patches [P, L] with l = (cin, ki, kj) order."""
    xpad = np.zeros((B, CIN, H + 2 * PAD, W + 2 * PAD), dtype=xi.dtype)
    xpad[:, :, PAD:PAD + H, PAD:PAD + W] = xi
    cols = [xpad[:, :, ki:ki + H, kj:kj + W]
            for ki in range(K) for kj in range(K)]
    p = np.stack(cols, axis=2)  # [B, CIN, K*K, H, W]
    return p.reshape(B, L, H * W).transpose(0, 2, 1).reshape(B * H * W, L)


def _prepare(x, w):
    """Quantize + stage inputs; returns (program_key, builder, in_maps)."""
    x = np.asarray(x, dtype=np.float32)
    w = np.asarray(w, dtype=np.float32)

    xi, wi = _quantize_inputs(x, w)          # int64: [B,CIN,H,W], [COUT, L]
    patches = _im2col(xi)                    # [P, L] int64
    wmat = wi.T                              # [L, COUT] int64

    # fp16 path is exact iff |w_int| fits fp16's 11-bit mantissa (the x split
    # parts 256*xh in [-2^15, 2^15) and xl in [0, 256) are always exact).
    use_fp16 = np.abs(wi).max() <= 2048

    if use_fp16:
        xh = patches >> 8                    # floor division: [-128, 128)
        xl = patches & 0xFF                  # [0, 256)
        xe = np.empty((2 * L, B * H * W), dtype=np.float16)
        xe[0::2, :] = (xh.T * 256).astype(np.float16)
        xe[1::2, :] = xl.T.astype(np.float16)
        we = np.repeat(wmat, 2, axis=0).astype(np.float16)   # [2L, COUT]
        xtiles = np.ascontiguousarray(xe.reshape(KT16, 128, B * H * W))
        wtiles = np.ascontiguousarray(we.reshape(KT16, 128, COUT))
        key = "nc16"
        builder = _build_fp16_program
    else:
        xe = np.zeros((KT32 * 128, B * H * W), dtype=np.float32)
        xe[:L, :] = patches.T.astype(np.float32)
        we = np.zeros((KT32 * 128, COUT), dtype=np.float32)
        we[:L, :] = wmat.astype(np.float32)
        xtiles = np.ascontiguousarray(xe.reshape(KT32, 128, B * H * W))
        wtiles = np.ascontiguousarray(we.reshape(KT32, 128, COUT))
        key = "nc32"
        builder = _build_fp32_program

    in_maps = []
    for core in range(N_CORES):
        p0 = core * PIX_PER_CORE
        in_maps.append({
            "xk": np.ascontiguousarray(xtiles[:, :, p0:p0 + PIX_PER_CORE]),
            "wk": wtiles,
        })
    return key, builder, in_maps


def kernel(x: np.ndarray, w: np.ndarray) -> np.ndarray:
    from concourse.bass_utils import run_bass_kernel_spmd

    key, builder, in_maps = _prepare(x, w)
    if key not in _CACHE:
        _CACHE[key] = builder()
    nc = _CACHE[key]

    results = run_bass_kernel_spmd(nc, in_maps, list(range(N_CORES))).results

    # Per-core shard: [128 pixels, COUT], pixels are (row, col) of half an image.
    out = np.empty((B, COUT, H, W), dtype=np.float32)
    for core in range(N_CORES):
        b, half = divmod(core, 2)
        r0 = half * ROWS_PER_CORE
        shard = results[core]["out"].reshape(ROWS_PER_CORE, W, COUT)
        out[b, :, r0:r0 + ROWS_PER_CORE, :] = shard.transpose(2, 0, 1)
    return out


# revision 7
# speedup vs baseline: 1.2768x; 1.0629x over previous
"""Trainium2 Bass kernel for nn_Conv2d_mvm (PUMA bit-sliced crossbar conv emulation).

Math identity
-------------
The reference emulates an analog crossbar MVM: inputs become 16-bit
two's-complement bit-streams, weights become 2-bit slices of the 16-bit
magnitudes of their pos/neg parts, and ADC = clip(round(analog), 0, 511).
Each analog column sum is at most 128*3 = 384 < 511 and every quantity is a
small exact integer held in f32, so the ADC is the identity and the whole
pipeline is linear in the bits/slices. Shift-add therefore reconstructs

    out[p, c] = quant( (x_int[p, :] . w_int[c, :]) / 2^24 )

with x_int = round(patch * 2^12) (int16 wrap),
w_int = clip(round(relu(w)*2^12), 0, 65535) - clip(round(relu(-w)*2^12), 0, 65535),
quant(v) = clip(round(v * 2^12), -2^15, 2^15-1) / 2^12  (round-half-even).

Device kernel
-------------
Data-parallel over the P = 1024 output pixels: each of 8 cores computes 128
pixels (half of one batch image) against the replicated [L=576, Cout=128]
integer weight matrix.

The PE's fp32 matmul is double-pumped (LOW/HIGH passes), so the integer
matmul is run in fp16 instead, which is exact here: |w_int| < 2048 fits
fp16's 11-bit mantissa, and x_int = 256*xh + xl splits into two fp16-exact
factors. The split folds into the contraction dimension,

    acc[p,c] = sum_l (256*xh[l,p]) * w[l,c] + xl[l,p] * w[l,c],

giving k = 2L = 1152 = 9 exact tiles of 128: a single accumulation group of
9 single-pass fp16 matmuls into one PSUM bank. A 3-instruction DVE epilogue
applies the fixed-point quantizer (RNE via the +1.5*2^23 magic constant,
clip, rescale). Should some input exceed the fp16-exact ranges, kernel()
falls back to an fp32 program (5 double-pumped matmuls over k = 640).
"""

import numpy as np

# Problem constants (hardcoded: kernel.py must be self-contained).
B, CIN, H, W = 4, 64, 16, 16
COUT = 128
K, PAD = 3, 1
IF = 12           # input frac bits
WF = 12           # weight frac bits
ACM_FRAC = 12
L = CIN * K * K   # 576
N_CORES = 8
ROWS_PER_CORE = H // 2            # 8 pixel rows per core
PIX_PER_CORE = ROWS_PER_CORE * W  # 128
KT16 = 2 * L // 128               # 9 fp16 k-tiles
KT32 = 5                          # fp32 k-tiles (640 = 5*128, zero-padded)

_CACHE = {}

_MAGIC = float(np.float32(1.5 * 2 ** 23))  # f32 RNE rounding constant
_INV_Q = 1.0 / (1 << ACM_FRAC)
_LO = float(-(1 << 15))
_HI = float((1 << 15) - 1)


def _epilogue_and_out(nc, mybir, pool, acc, out):
    """q = clip(round(acc / 2^12), -2^15, 2^15-1) / 2^12, then store."""
    res = pool.tile([PIX_PER_CORE, COUT], mybir.dt.float32, name="res")
    nc.vector.tensor_scalar(res[:, :], acc[:, :], _INV_Q, _MAGIC,
                            op0=mybir.AluOpType.mult,
                            op1=mybir.AluOpType.add)
    nc.vector.tensor_scalar(res[:, :], res[:, :], _MAGIC, _LO,
                            op0=mybir.AluOpType.subtract,
                            op1=mybir.AluOpType.max)
    nc.vector.tensor_scalar(res[:, :], res[:, :], _HI, _INV_Q,
                            op0=mybir.AluOpType.min,
                            op1=mybir.AluOpType.mult)
    nc.sync.dma_start(out, res[:, :])


def _build_fp16_program():
    """9 single-pass fp16 matmuls; inputs are pre-tiled [9, 128, 128] arrays."""
    import concourse.bacc as bacc
    import concourse.mybir as mybir
    import concourse.tile as tile

    nc = bacc.Bacc("TRN2", target_bir_lowering=False, debug=False,
                   num_devices=N_CORES)
    xk = nc.dram_tensor("xk", [128, KT16, PIX_PER_CORE], mybir.dt.float16,
                        kind="ExternalInput").ap()
    wk = nc.dram_tensor("wk", [128, KT16, COUT], mybir.dt.float16,
                        kind="ExternalInput").ap()
    out = nc.dram_tensor("out", [PIX_PER_CORE, COUT], mybir.dt.float32,
                         kind="ExternalOutput").ap()

    with tile.TileContext(nc) as tc:
        with (
            tc.tile_pool(name="sbuf", bufs=1) as pool,
            tc.tile_pool(name="psum", bufs=1, space="PSUM") as psum_pool,
        ):
            xt = pool.tile([128, KT16, PIX_PER_CORE], mybir.dt.float16, name="xt")
            wt = pool.tile([128, KT16, COUT], mybir.dt.float16, name="wt")
            # Two large DMAs on different queues so their issue overlaps.
            nc.sync.dma_start(xt[:, :, :], xk[:, :, :])
            nc.gpsimd.dma_start(wt[:, :, :], wk[:, :, :])

            acc = psum_pool.tile([PIX_PER_CORE, COUT], mybir.dt.float32,
                                 name="acc")
            for r in range(KT16):
                nc.tensor.matmul(acc[:, :], xt[:, r, :], wt[:, r, :],
                                 start=(r == 0), stop=(r == KT16 - 1))
            _epilogue_and_out(nc, mybir, pool, acc, out)

    nc.compile()
    return nc


def _build_fp32_program():
    """Fallback: 5 double-pumped fp32 matmuls over zero-padded k = 640."""
    import concourse.bacc as bacc
    import concourse.mybir as mybir
    import concourse.tile as tile

    nc = bacc.Bacc("TRN2", target_bir_lowering=False, debug=False,
                   num_devices=N_CORES)
    xk = nc.dram_tensor("xk", [128, KT32, PIX_PER_CORE], mybir.dt.float32,
                        kind="ExternalInput").ap()
    wk = nc.dram_tensor("wk", [128, KT32, COUT], mybir.dt.float32,
                        kind="ExternalInput").ap()
    out = nc.dram_tensor("out", [PIX_PER_CORE, COUT], mybir.dt.float32,
                         kind="ExternalOutput").ap()

    with tile.TileContext(nc) as tc:
        with (
            tc.tile_pool(name="sbuf", bufs=1) as pool,
            tc.tile_pool(name="psum", bufs=1, space="PSUM") as psum_pool,
        ):
            xt = pool.tile([128, KT32, PIX_PER_CORE], mybir.dt.float32, name="xt")
            wt = pool.tile([128, KT32, COUT], mybir.dt.float32, name="wt")
            nc.sync.dma_start(xt[:, :, :], xk[:, :, :])
            nc.gpsimd.dma_start(wt[:, :, :], wk[:, :, :])

            acc = psum_pool.tile([PIX_PER_CORE, COUT], mybir.dt.float32,
                                 name="acc")
            for r in range(KT32):
                nc.tensor.matmul(acc[:, :], xt[:, r, :], wt[:, r, :],
                                 start=(r == 0), stop=(r == KT32 - 1))
            _epilogue_and_out(nc, mybir, pool, acc, out)

    nc.compile()
    return nc


def _quantize_inputs(x, w):
    """Reproduce the reference's fixed-point quantization bit-exactly."""
    xi = np.round(x.astype(np.float32) * (1 << IF)).astype(np.int64)
    xi = ((xi + (1 << 15)) & 0xFFFF) - (1 << 15)  # int16 two's-complement wrap

    wf = w.reshape(COUT, L).astype(np.float32)
    w_pos = np.clip(np.round(np.clip(wf, 0, None) * (1 << WF)), 0, 65535)
    w_neg = np.clip(np.round(np.abs(np.clip(wf, None, 0)) * (1 << WF)), 0, 65535)
    wi = (w_pos - w_neg).astype(np.int64)  # [COUT, L], l = (cin, ki, kj)
    return xi, wi


def _im2col(xi):
    """[B, CIN, H, W] int -> patches [P, L] with l = (cin, ki, kj) order."""
    xpad = np.zeros((B, CIN, H + 2 * PAD, W + 2 * PAD), dtype=xi.dtype)
    xpad[:, :, PAD:PAD + H, PAD:PAD + W] = xi
    cols = [xpad[:, :, ki:ki + H, kj:kj + W]
            for ki in range(K) for kj in range(K)]
    p = np.stack(cols, axis=2)  # [B, CIN, K*K, H, W]
    return p.reshape(B, L, H * W).transpose(0, 2, 1).reshape(B * H * W, L)


def _prepare(x, w):
    """Quantize + stage inputs; returns (program_key, builder, in_maps)."""
    x = np.asarray(x, dtype=np.float32)
    w = np.asarray(w, dtype=np.float32)

    xi, wi = _quantize_inputs(x, w)          # int64: [B,CIN,H,W], [COUT, L]
    patches = _im2col(xi)                    # [P, L] int64
    wmat = wi.T                              # [L, COUT] int64

    # fp16 path is exact iff |w_int| fits fp16's 11-bit mantissa (the x split
    # parts 256*xh in [-2^15, 2^15) and xl in [0, 256) are always exact).
    use_fp16 = np.abs(wi).max() <= 2048

    if use_fp16:
        xh = patches >> 8                    # floor division: [-128, 128)
        xl = patches & 0xFF                  # [0, 256)
        xe = np.empty((2 * L, B * H * W), dtype=np.float16)
        xe[0::2, :] = (xh.T * 256).astype(np.float16)
        xe[1::2, :] = xl.T.astype(np.float16)
        we = np.repeat(wmat, 2, axis=0).astype(np.float16)   # [2L, COUT]
        xtiles = np.ascontiguousarray(
            xe.reshape(KT16, 128, B * H * W).transpose(1, 0, 2))
        wtiles = np.ascontiguousarray(
            we.reshape(KT16, 128, COUT).transpose(1, 0, 2))
        key = "nc16"
        builder = _build_fp16_program
    else:
        xe = np.zeros((KT32 * 128, B * H * W), dtype=np.float32)
        xe[:L, :] = patches.T.astype(np.float32)
        we = np.zeros((KT32 * 128, COUT), dtype=np.float32)
        we[:L, :] = wmat.astype(np.float32)
        xtiles = np.ascontiguousarray(
            xe.reshape(KT32, 128, B * H * W).transpose(1, 0, 2))
        wtiles = np.ascontiguousarray(
            we.reshape(KT32, 128, COUT).transpose(1, 0, 2))
        key = "nc32"
        builder = _build_fp32_program

    in_maps = []
    for core in range(N_CORES):
        p0 = core * PIX_PER_CORE
        in_maps.append({
            "xk": np.ascontiguousarray(xtiles[:, :, p0:p0 + PIX_PER_CORE]),
            "wk": wtiles,
        })
    return key, builder, in_maps


def kernel(x: np.ndarray, w: np.ndarray) -> np.ndarray:
    from concourse.bass_utils import run_bass_kernel_spmd

    key, builder, in_maps = _prepare(x, w)
    if key not in _CACHE:
        _CACHE[key] = builder()
    nc = _CACHE[key]

    results = run_bass_kernel_spmd(nc, in_maps, list(range(N_CORES))).results

    # Per-core shard: [128 pixels, COUT], pixels are (row, col) of half an image.
    out = np.empty((B, COUT, H, W), dtype=np.float32)
    for core in range(N_CORES):
        b, half = divmod(core, 2)
        r0 = half * ROWS_PER_CORE
        shard = results[core]["out"].reshape(ROWS_PER_CORE, W, COUT)
        out[b, :, r0:r0 + ROWS_PER_CORE, :] = shard.transpose(2, 0, 1)
    return out
